# revision 26
# baseline (speedup 1.0000x reference)
"""Causal attention (naive double-normalize reference == causal softmax) on 8 TRN2 cores.

v2: fp8 DoubleRow for the score and AV phases (the projections stay bf16 for
accuracy), plus collective restructuring.

Math: scores = (x Wq)(x Wk)^T = (x Wqk) x^T with Wqk = Wq Wk^T folded on the
host. Q~ = x_q Wqk and V = x_kv Wv are computed in bf16 (fp8 projections fail
the error budget). Q~ is then stored fp8 and contracted against host-prepped
fp8 x^T tiles with DoubleRow matmuls (2 MACs/cell/cycle). P = exp(scale*s - 2)
is stored fp8 (the -2 bias keeps exp under the e4m3 240 max; it cancels in the
row normalization). V is bounced and AllGathered
directly in fp8 (halving collective wire time) and AV runs DoubleRow too.

fp8 score noise is too large for rows with few causal keys (no averaging), so
the first 512 global rows are recomputed with bf16 scores ("fixup"): bf16 Q~
slice x bf16 x^T[0:512] -> bf16 P -> AV against bf16 V[0:128] (recomputed
locally on every core during vproj; those rows only exist on core 0) plus fp8
V[128:512] from the gather, overwriting the first 64 local output rows.

Collectives: the one-time rendezvous barrier (~21->76us) is runtime-paced and
cannot be absorbed; each collective op costs ~17us fixed on top of wire time,
so V goes in two fp8 1024-column halves fired at vproj's midpoint and end.

Sharding (unchanged): Q rows interleaved (core i owns global rows {8l+i}),
V rows contiguous (core i computes rows [512i, 512(i+1))).
"""

import math

import numpy as np

D = 2048          # d_in == d_out
CC = D // 128     # contraction chunks (16)
DT = D // 128     # output d tiles (16)
N_CORES = 8
FIX = 512         # global rows recomputed in bf16 (per-core local m < FIX//8)

_BUILT = {}


def _build(S):
    import concourse.bacc as bacc
    import concourse.mybir as mybir
    import concourse.tile as tile

    f32 = mybir.dt.float32
    bf16 = mybir.dt.bfloat16
    f8 = mybir.dt.float8e4
    ML = S // N_CORES          # local q rows per core (512)
    MF = FIX // N_CORES        # local q rows covered by the fixup (64)
    NH = ML // 128             # output row tiles per core (4)
    NJ = S // 128              # key tiles (32)
    NJF = FIX // 128           # key tiles covered by the fixup (4)
    SCALE = 1.0 / math.sqrt(D)
    EXP = mybir.ActivationFunctionType.Exp
    CPY = mybir.ActivationFunctionType.Copy
    DR = mybir.MatmulPerfMode.DoubleRow
    RG = [list(range(N_CORES))]
    XT_BUFS = 16               # all xT tiles resident (fp8 halves them)
    DR_J = 22                  # score key tiles using DoubleRow (N>=176); tail
    #                            uses fp8 normal mode (FWL) where DR's 256-col
    #                            LDWEIGHTS would dominate the short stream

    nc = bacc.Bacc("TRN2", target_bir_lowering=False)

    xq = nc.declare_dram_parameter("xq", [128, CC, ML], bf16, isOutput=False)
    xkv = nc.declare_dram_parameter("xkv", [128, CC, ML], bf16, isOutput=False)
    xt = nc.declare_dram_parameter("xt", [128, CC, S], f8, isOutput=False)
    xt0b = nc.declare_dram_parameter("xt0b", [128, CC, FIX], bf16, isOutput=False)
    wqk = nc.declare_dram_parameter("wqk", [DT, 128, CC, 128], bf16, isOutput=False)
    wv = nc.declare_dram_parameter("wv", [128, CC, D], bf16, isOutput=False)
    maskp = nc.declare_dram_parameter("mask", [128, 16], bf16, isOutput=False)
    maskp8 = nc.declare_dram_parameter("mask8", [128, 16], f8, isOutput=False)
    out = nc.declare_dram_parameter("out", [ML, D], f32, isOutput=True)

    with tile.TileContext(nc) as tc:
        with (
            tc.tile_pool(name="const", bufs=1) as const,
            tc.tile_pool(name="dram", bufs=1, space="DRAM") as dram,
            tc.tile_pool(name="xtstream", bufs=XT_BUFS) as xtstream,
        ):
            qt8_sb = const.tile([128, CC, ML], f8)
            qt16_sb = const.tile([128, CC, MF], bf16)
            xt0_sb = const.tile([128, CC, FIX], bf16)
            mask_sb = const.tile([128, 16], bf16)
            mask8_sb = const.tile([128, 16], f8)
            ones_sb = const.tile([128, 1], bf16)
            ones8_sb = const.tile([128, 2, 16], f8)
            one1_sb = const.tile([1, 1], f32)
            rs_sb = const.tile([1, ML], f32)
            rsf_sb = const.tile([1, MF], f32)
            rin_sb = const.tile([128, NH], f32)
            recip_sb = const.tile([128, NH], f32)
            rinf_sb = const.tile([MF, 1], f32)
            recf_sb = const.tile([MF, 1], f32)
            pf_sb = const.tile([128, NJF, MF], bf16)
            pf8_sb = const.tile([128, NJF, MF], f8)
            v0_sb = const.tile([128, D], bf16)
            outf_sb = const.tile([MF, 4, 512], f32)

            warm_sb = const.tile([1, 1], f32)
            bias_sb = const.tile([128, 1], f32)
            nc.vector.memset(bias_sb[:], -2.0)
            nc.vector.memset(ones_sb[:], 1.0)
            nc.vector.memset(ones8_sb[:], 1.0)
            nc.vector.memset(one1_sb[:], 1.0)
            # Load the Exp activation table while the PE warms up.
            nc.scalar.activation(
                out=warm_sb[:], in_=one1_sb[:],
                func=mybir.ActivationFunctionType.Exp,
            )
            nc.scalar.dma_start(out=mask_sb[:], in_=maskp[:])
            nc.scalar.dma_start(out=mask8_sb[:], in_=maskp8[:])

            # xt0 (bf16 x^T of the first FIX keys) loads on the scalar ring
            # ahead of the bounce stores: the v0 recompute consumes it
            # mid-vproj. (Not at t=0 on gpsimd - an early bulk load starves
            # the xkv/wv critical path.)
            nc.scalar.dma_start(out=xt0_sb[:], in_=xt0b[:])

            # V bounced and gathered in fp8, in two 1024-column halves
            # (each collective carries ~17us fixed overhead, so halves beat
            # quarters once the payload is fp8).
            v_bounce = [
                dram.tile([ML, 1024], f8, name=f"vb{i}") for i in range(2)
            ]
            v_ag = [
                dram.tile([S, 1024], f8, addr_space="Shared", name=f"vag{i}")
                for i in range(2)
            ]

            xt_tiles = {}

            def load_xt(tdx, eng):
                xtp = xtstream.tile([128, CC, 256], f8, tag="xt", name=f"xt{tdx}")
                eng.dma_start(out=xtp[:], in_=xt[:, :, 256 * tdx:256 * (tdx + 1)])
                xt_tiles[tdx] = xtp

            # ============ projections ============
            with (
                tc.tile_pool(name="px", bufs=1) as px,
                tc.tile_pool(name="stage", bufs=4) as stage,
                tc.tile_pool(name="proj_ps", bufs=4, space="PSUM") as proj_ps,
                tc.tile_pool(name="wvhold", bufs=4) as wvhold,
                tc.tile_pool(name="wqstream", bufs=8) as wqstream,
            ):
                xkv_sb = px.tile([128, CC, ML], bf16)
                xq_sb = px.tile([128, CC, ML], bf16)

                def load_wv(wc, wv_tiles):
                    wvt = wvhold.tile(
                        [128, CC, 256], bf16, tag="wv", name=f"wv{wc}"
                    )
                    nc.sync.dma_start(
                        out=wvt[:], in_=wv[:, :, 256 * wc:256 * (wc + 1)]
                    )
                    wv_tiles.append(wvt)

                # first-MM critical path: xkv chunk 0 + wv0 lead the ring
                nc.sync.dma_start(out=xkv_sb[:, 0:4, :], in_=xkv[:, 0:4, :])
                wv_tiles = []
                load_wv(0, wv_tiles)
                nc.sync.dma_start(out=xkv_sb[:, 4:CC, :], in_=xkv[:, 4:CC, :])
                load_wv(1, wv_tiles)

                # ---- V projection -> fp8 bounce -> half AllGathers ----
                # Also recomputes V[0:128] in bf16 for every core (the fixup
                # needs accurate V for the earliest keys; those rows live
                # only on core 0, and collectives can't broadcast them).
                with tc.spectator_scope("vproj"):
                    for cs in range(4):
                        for half in range(2):
                            wc = 2 * cs + half
                            if wc >= 2:
                                load_wv(wc, wv_tiles)
                        for nt in range(ML // 128):
                            st = stage.tile(
                                [128, 512], f8, tag="stage",
                                name=f"vst{cs}_{nt}",
                            )
                            for half in range(2):
                                wvt = wv_tiles[2 * cs + half]
                                ps = proj_ps.tile(
                                    [128, ML], f32, tag="proj",
                                    name=f"vps{cs}_{nt}_{half}",
                                )
                                for c in range(CC):
                                    nc.tensor.matmul(
                                        out=ps[:, 0:256],
                                        lhsT=xkv_sb[:, c, 128 * nt:128 * (nt + 1)],
                                        rhs=wvt[:, c, :],
                                        start=(c == 0), stop=(c == CC - 1),
                                    )
                                nc.vector.tensor_copy(
                                    out=st[:, 256 * half:256 * (half + 1)],
                                    in_=ps[:, 0:256],
                                )
                            nc.scalar.dma_start(
                                out=v_bounce[cs // 2][128 * nt:128 * (nt + 1),
                                                     512 * (cs % 2):
                                                     512 * (cs % 2) + 512],
                                in_=st[:],
                            )
                        # bf16 V[0:128] for this column block (reuses the
                        # live wv tiles and the early-loaded xt0)
                        for half in range(2):
                            wc = 2 * cs + half
                            wvt = wv_tiles[wc]
                            ps = proj_ps.tile(
                                [128, ML], f32, tag="proj", name=f"v0ps{wc}"
                            )
                            for c in range(CC):
                                nc.tensor.matmul(
                                    out=ps[:, 0:256],
                                    lhsT=xt0_sb[:, c, 0:128],
                                    rhs=wvt[:, c, :],
                                    start=(c == 0), stop=(c == CC - 1),
                                )
                            nc.vector.tensor_copy(
                                out=v0_sb[:, 256 * wc:256 * (wc + 1)],
                                in_=ps[:, 0:256],
                            )
                        if cs % 2 == 1:
                            nc.gpsimd.collective_compute(
                                "AllGather", mybir.AluOpType.bypass,
                                replica_groups=RG,
                                ins=[v_bounce[cs // 2][:].opt()],
                                outs=[v_ag[cs // 2][:].opt()],
                            )

                # ---- Q~ projection (bf16 in, fp8 + small bf16 out) ----
                nc.sync.dma_start(out=xq_sb[:], in_=xq[:])
                wq_tiles = []

                def load_wq(dt):
                    wqt = wqstream.tile(
                        [128, CC, 128], bf16, tag="wq", name=f"wq{dt}"
                    )
                    nc.sync.dma_start(out=wqt[:], in_=wqk[dt])
                    wq_tiles.append(wqt)

                # Sync-ring order: xt's 8MB goes BEFORE wqk so it clears
                # the wires before the AllGather window (the wqk stream is
                # consumed at only ~1MB/4us, so it tolerates AG congestion;
                # the wqstream pool's 8-buf rotation self-throttles it).
                for tdx in range(XT_BUFS):
                    load_xt(tdx, nc.sync)
                for dt in range(DT):
                    load_wq(dt)
                with tc.spectator_scope("qtproj"):
                    for dt in range(DT):
                        ps = proj_ps.tile(
                            [128, ML], f32, tag="proj", name=f"qps{dt}"
                        )
                        for c in range(CC):
                            nc.tensor.matmul(
                                out=ps[:], lhsT=wq_tiles[dt][:, c, :],
                                rhs=xq_sb[:, c, :],
                                start=(c == 0), stop=(c == CC - 1),
                            )
                        nc.vector.tensor_copy(out=qt8_sb[:, dt, :], in_=ps[:])
                        nc.vector.tensor_copy(
                            out=qt16_sb[:, dt, :], in_=ps[:, 0:MF]
                        )

            # ============ attention ============
            with (
                tc.tile_pool(name="attn", bufs=1) as attn,
                tc.tile_pool(name="v8pool", bufs=8) as v8pool,
                tc.tile_pool(name="vfixp", bufs=4) as vfixp,
                tc.tile_pool(name="avstage", bufs=4) as avstage,
                tc.tile_pool(name="outp", bufs=4) as outp,
                tc.tile_pool(name="st_ps", bufs=2, space="PSUM") as st_ps,
                tc.tile_pool(name="rs_ps", bufs=1, space="PSUM") as rs_ps,
                tc.tile_pool(name="av_ps", bufs=1, space="PSUM") as av_ps,
                tc.tile_pool(name="tp_ps", bufs=1, space="PSUM") as tp_ps,
            ):
                p_all = attn.tile([128, NJ, ML], f8)
                # fp8 V rows 128:FIX per 512-column block, for the fixup's
                # later key tiles (rows 0:128 use the bf16 v0 recompute).
                # Head-blocked on each half's gather.
                vfix_tiles = []
                for ct in range(4):
                    vf = vfixp.tile(
                        [128, NJF - 1, 512], f8, tag="vfix", name=f"vfix{ct}"
                    )
                    c0 = 512 * (ct % 2)
                    nc.scalar.dma_start(
                        out=vf[:],
                        in_=v_ag[ct // 2][128:FIX, c0:c0 + 512]
                        .rearrange("(jj p) n -> p jj n", p=128),
                    )
                    vfix_tiles.append(vf)

                # ---- fixup scores: bf16 recompute of rows g < FIX ----
                with tc.spectator_scope("fixsc"):
                    for jt in range(NJF):
                        psf = st_ps.tile([128, ML], f32, tag="st")
                        for c in range(CC):
                            nc.tensor.matmul(
                                out=psf[:, 0:MF],
                                lhsT=xt0_sb[:, c, 128 * jt:128 * (jt + 1)],
                                rhs=qt16_sb[:, c, :],
                                start=(c == 0), stop=(c == CC - 1),
                            )
                        pfj = pf_sb[:, jt, :]
                        m0f = 16 * jt
                        nc.scalar.activation(
                            out=pfj[:, m0f:MF], in_=psf[:, m0f:MF], func=EXP,
                            scale=SCALE,
                        )
                        nc.vector.tensor_tensor(
                            out=pfj[:, m0f:m0f + 16], in0=pfj[:, m0f:m0f + 16],
                            in1=mask_sb[:], op=mybir.AluOpType.mult,
                        )
                        if m0f > 0:
                            nc.vector.memset(pfj[:, 0:m0f], 0.0)
                    # fixup row sums -> transpose -> reciprocal
                    rsf = rs_ps.tile([1, ML], f32, tag="rs")
                    for jt in range(NJF):
                        m0f = 16 * jt
                        nc.tensor.matmul(
                            out=rsf[0:1, m0f:MF], lhsT=ones_sb[:],
                            rhs=pf_sb[:, jt, m0f:MF],
                            start=(jt == 0), stop=(jt == NJF - 1),
                        )
                    nc.scalar.activation(
                        out=rsf_sb[:], in_=rsf[0:1, 0:MF], func=CPY
                    )
                    tpf = tp_ps.tile([MF, 1], f32, tag="tp")
                    nc.tensor.matmul(
                        out=tpf[:], lhsT=rsf_sb[:], rhs=one1_sb[:],
                        start=True, stop=True,
                    )
                    nc.scalar.activation(out=rinf_sb[:], in_=tpf[:], func=CPY)
                    nc.vector.reciprocal(out=recf_sb[:], in_=rinf_sb[:])
                    # fp8 copy of the fixup P for the fp8-V key tiles
                    nc.vector.tensor_copy(out=pf8_sb[:], in_=pf_sb[:])

                # ---- main scores: fp8, DoubleRow over contraction pairs.
                # The rowsum matmuls interleave pair-by-pair so the renorm
                # has no serial tail after the last score tile. ----
                rs = rs_ps.tile([1, ML], f32, tag="rs")
                with tc.spectator_scope("scores"):
                    for j in range(NJ):
                        tdx = j // 2
                        s0 = 128 * (j % 2)
                        kt = xt_tiles[tdx][:, :, s0:s0 + 128]
                        m0 = 16 * j
                        ps = st_ps.tile([128, ML], f32, tag="st")
                        if j < DR_J:
                            for cp in range(CC // 2):
                                nc.tensor.matmul(
                                    out=ps[:, m0:ML],
                                    lhsT=kt[:, 2 * cp:2 * cp + 2, :],
                                    rhs=qt8_sb[:, 2 * cp:2 * cp + 2, m0:ML],
                                    start=(cp == 0), stop=(cp == CC // 2 - 1),
                                    perf_mode=DR,
                                )
                        else:
                            for c in range(CC):
                                nc.tensor.matmul(
                                    out=ps[:, m0:ML], lhsT=kt[:, c, :],
                                    rhs=qt8_sb[:, c, m0:ML],
                                    start=(c == 0), stop=(c == CC - 1),
                                )
                        pj = p_all[:, j, :]
                        # -2 bias keeps exp(s) under the e4m3 max (240); it
                        # cancels in the row normalization.
                        nc.scalar.activation(
                            out=pj[:, m0:ML], in_=ps[:, m0:ML], func=EXP,
                            scale=SCALE, bias=bias_sb[:],
                        )
                        nc.vector.tensor_tensor(
                            out=pj[:, m0:m0 + 16], in0=pj[:, m0:m0 + 16],
                            in1=mask8_sb[:], op=mybir.AluOpType.mult,
                        )
                        g0 = 128 * (j // 8)
                        if m0 > g0:
                            nc.vector.memset(pj[:, g0:m0], 0.0)
                        if j % 2 == 1:
                            j0 = j - 1
                            m0p = 16 * j0
                            nc.tensor.matmul(
                                out=rs[0:1, m0p:ML], lhsT=ones8_sb[:, :, 0:1],
                                rhs=p_all[:, j0:j0 + 2, m0p:ML],
                                start=(j0 == 0), stop=(j0 == NJ - 2),
                                perf_mode=DR,
                            )


                # ---- fixup AV: after the scores so its ACT drains sit
                # behind the exp stream (they gate only the main AV's first
                # PSUM bank, which starts later anyway) ----
                with tc.spectator_scope("fixav"):
                    for cs in range(4):
                        av_f = av_ps.tile(
                            [128, 512], f32, tag="av0", name=f"avf_{cs}"
                        )
                        nc.tensor.matmul(
                            out=av_f[0:MF, :], lhsT=pf_sb[:, 0, :],
                            rhs=v0_sb[:, 512 * cs:512 * (cs + 1)],
                            start=True, stop=False,
                        )
                        for jt in range(1, NJF):
                            nc.tensor.matmul(
                                out=av_f[0:MF, :], lhsT=pf8_sb[:, jt, :],
                                rhs=vfix_tiles[cs][:, jt - 1, :],
                                start=False, stop=(jt == NJF - 1),
                            )
                        nc.scalar.activation(
                            out=outf_sb[:, cs, :], in_=av_f[0:MF, :], func=CPY,
                            scale=recf_sb[:],
                        )

                with tc.spectator_scope("renorm"):
                    nc.scalar.activation(out=rs_sb[:], in_=rs[:], func=CPY)
                    for h in range(NH):
                        tp = tp_ps.tile([128, 1], f32, tag="tp")
                        nc.tensor.matmul(
                            out=tp[:], lhsT=rs_sb[0:1, 128 * h:128 * (h + 1)],
                            rhs=one1_sb[:], start=True, stop=True,
                        )
                        nc.scalar.activation(
                            out=rin_sb[:, h:h + 1], in_=tp[:], func=CPY
                        )
                    nc.vector.reciprocal(out=recip_sb[:], in_=rin_sb[:])

                with tc.spectator_scope("av"):
                    for cs in range(4):
                        av = [
                            av_ps.tile([128, 512], f32, tag=f"av{h}",
                                       name=f"av{h}_{cs}")
                            for h in range(NH)
                        ]
                        c0 = 512 * (cs % 2)
                        for t in range((NJ + 3) // 4):
                            vt8 = v8pool.tile([128, 4, 512], f8, tag="v8")
                            nc.sync.dma_start(
                                out=vt8[:],
                                in_=v_ag[cs // 2][512 * t:512 * (t + 1),
                                                  c0:c0 + 512]
                                .rearrange("(jj p) n -> p jj n", p=128),
                            )
                            for jj0 in (0, 2):
                                j0 = 4 * t + jj0
                                for h in range(j0 // 8, NH):
                                    nc.tensor.matmul(
                                        out=av[h][:],
                                        lhsT=p_all[:, j0:j0 + 2,
                                                   128 * h:128 * (h + 1)],
                                        rhs=vt8[:, jj0:jj0 + 2, :],
                                        start=(j0 == 0),
                                        stop=(j0 + 1 == min(8 * (h + 1), NJ) - 1),
                                        perf_mode=DR,
                                    )
                            if t % 2 == 1:
                                h = (t - 1) // 2
                                stg = avstage.tile(
                                    [128, 512], f32, tag="avs",
                                    name=f"avs{h}_{cs}"
                                )
                                nc.vector.tensor_copy(out=stg[:], in_=av[h][:])
                                ob = outp.tile([128, 512], f32, tag="out")
                                nc.scalar.activation(
                                    out=ob[:], in_=stg[:], func=CPY,
                                    scale=recip_sb[:, h:h + 1],
                                )
                                if h == 0:
                                    # overwrite the fixed-up early rows
                                    nc.vector.tensor_copy(
                                        out=ob[0:MF, :], in_=outf_sb[:, cs, :]
                                    )
                                nc.scalar.dma_start(
                                    out=out[128 * h:128 * (h + 1),
                                            512 * cs:512 * (cs + 1)],
                                    in_=ob[:],
                                )

    nc.finalize()
    return nc


def _prep_inputs(x, Wq, Wk, Wv, S):
    import ml_dtypes

    bf = ml_dtypes.bfloat16
    f8 = ml_dtypes.float8_e4m3
    ML = S // N_CORES

    def shuf_w(W):
        # [dt, p, c, j] layout: element = W[128c+p, 128dt+j]
        return np.ascontiguousarray(
            W.reshape(CC, 128, DT, 128).transpose(2, 1, 0, 3)
        ).astype(bf)

    wqk_h = shuf_w((Wq @ Wk.T).astype(np.float32))
    wv_h = np.ascontiguousarray(
        Wv.reshape(CC, 128, D).transpose(1, 0, 2)
    ).astype(bf)

    def shuf_x(rows, dt):
        # rows [n, D] -> [p, c, m] with element = rows[m, 128c+p]
        n = rows.shape[0]
        return np.ascontiguousarray(
            rows.reshape(n, CC, 128).transpose(2, 1, 0)
        ).astype(dt)

    xt_h = shuf_x(x, f8)
    xt0b_h = shuf_x(x[0:FIX], bf)
    in_maps = []
    for i in range(N_CORES):
        mask = (np.arange(128)[:, None] <= 8 * np.arange(16)[None, :] + i)
        in_maps.append({
            "xq": shuf_x(x[i::N_CORES], bf),
            "xkv": shuf_x(x[ML * i:ML * (i + 1)], bf),
            "xt": xt_h,
            "xt0b": xt0b_h,
            "wqk": wqk_h, "wv": wv_h,
            "mask": mask.astype(bf),
            "mask8": mask.astype(f8),
        })
    return in_maps


def run(x, Wq, Wk, Wv, S, trace=False, trace_cores=None):
    from concourse.bass_utils import run_bass_kernel_spmd

    if S not in _BUILT:
        _BUILT[S] = _build(S)
    nc = _BUILT[S]
    in_maps = _prep_inputs(x, Wq, Wk, Wv, S)
    res = run_bass_kernel_spmd(
        nc, in_maps, list(range(N_CORES)), trace=trace, trace_cores=trace_cores
    )
    outs = [res.results[i]["out"] for i in range(N_CORES)]
    full = np.stack(outs, axis=1).reshape(S, D).astype(np.float32)
    return full, res


def kernel(x, Wq, Wk, Wv):
    x = np.asarray(x, dtype=np.float32)
    Wq = np.asarray(Wq, dtype=np.float32)
    Wk = np.asarray(Wk, dtype=np.float32)
    Wv = np.asarray(Wv, dtype=np.float32)
    full, _ = run(x, Wq, Wk, Wv, x.shape[0])
    return full


# revision 27
# speedup vs baseline: 1.0978x; 1.0978x over previous
"""Causal attention (naive double-normalize reference == causal softmax) on 8 TRN2 cores.

v2: fp8 DoubleRow for the score and AV phases (the projections stay bf16 for
accuracy), plus collective restructuring.

Math: scores = (x Wq)(x Wk)^T = (x Wqk) x^T with Wqk = Wq Wk^T folded on the
host. Q~ = x_q Wqk and V = x_kv Wv are computed in bf16 (fp8 projections fail
the error budget). Q~ is then stored fp8 and contracted against host-prepped
fp8 x^T tiles with DoubleRow matmuls (2 MACs/cell/cycle). P = exp(scale*s - 2)
is stored fp8 (the -2 bias keeps exp under the e4m3 240 max; it cancels in the
row normalization). V is bounced and AllGathered
directly in fp8 (halving collective wire time) and AV runs DoubleRow too.

fp8 score noise is too large for rows with few causal keys (no averaging), so
the first 512 global rows are recomputed with bf16 scores ("fixup"): bf16 Q~
slice x bf16 x^T[0:512] -> bf16 P -> AV against bf16 V[0:128] (recomputed
locally on every core during vproj; those rows only exist on core 0) plus fp8
V[128:512] from the gather, overwriting the first 64 local output rows.

Collectives: the one-time rendezvous barrier (~21->76us) is runtime-paced and
cannot be absorbed; each collective op costs ~17us fixed on top of wire time,
so V goes in two fp8 1024-column halves fired at vproj's midpoint and end.

Sharding (unchanged): Q rows interleaved (core i owns global rows {8l+i}),
V rows contiguous (core i computes rows [512i, 512(i+1))).
"""

import math

import numpy as np

D = 2048          # d_in == d_out
CC = D // 128     # contraction chunks (16)
DT = D // 128     # output d tiles (16)
N_CORES = 8
FIX = 512         # global rows recomputed in bf16 (per-core local m < FIX//8)

_BUILT = {}


def _build(S):
    import concourse.bacc as bacc
    import concourse.mybir as mybir
    import concourse.tile as tile

    f32 = mybir.dt.float32
    bf16 = mybir.dt.bfloat16
    f8 = mybir.dt.float8e4
    ML = S // N_CORES          # local q rows per core (512)
    MF = FIX // N_CORES        # local q rows covered by the fixup (64)
    NH = ML // 128             # output row tiles per core (4)
    NJ = S // 128              # key tiles (32)
    NJF = FIX // 128           # key tiles covered by the fixup (4)
    SCALE = 1.0 / math.sqrt(D)
    EXP = mybir.ActivationFunctionType.Exp
    CPY = mybir.ActivationFunctionType.Copy
    DR = mybir.MatmulPerfMode.DoubleRow
    RG = [list(range(N_CORES))]
    XT_BUFS = 16               # all xT tiles resident (fp8 halves them)
    DR_J = 22                  # score key tiles using DoubleRow (N>=176); tail
    #                            uses fp8 normal mode (FWL) where DR's 256-col
    #                            LDWEIGHTS would dominate the short stream

    nc = bacc.Bacc("TRN2", target_bir_lowering=False)

    xq = nc.declare_dram_parameter("xq", [128, CC, ML], bf16, isOutput=False)
    xkv = nc.declare_dram_parameter("xkv", [128, CC, ML], bf16, isOutput=False)
    xt = nc.declare_dram_parameter("xt", [128, CC, S], f8, isOutput=False)
    xt0b = nc.declare_dram_parameter("xt0b", [128, CC, FIX], bf16, isOutput=False)
    wqk = nc.declare_dram_parameter("wqk", [DT, 128, CC, 128], bf16, isOutput=False)
    wv = nc.declare_dram_parameter("wv", [128, CC, D], bf16, isOutput=False)
    maskp = nc.declare_dram_parameter("mask", [128, 16], bf16, isOutput=False)
    maskp8 = nc.declare_dram_parameter("mask8", [128, 16], f8, isOutput=False)
    out = nc.declare_dram_parameter("out", [ML, D], f32, isOutput=True)

    with tile.TileContext(nc) as tc:
        with (
            tc.tile_pool(name="const", bufs=1) as const,
            tc.tile_pool(name="dram", bufs=1, space="DRAM") as dram,
            tc.tile_pool(name="xtstream", bufs=XT_BUFS) as xtstream,
        ):
            qt8_sb = const.tile([128, CC, ML], f8)
            qt16_sb = const.tile([128, CC, MF], bf16)
            xt0_sb = const.tile([128, CC, FIX], bf16)
            mask_sb = const.tile([128, 16], bf16)
            mask8_sb = const.tile([128, 16], f8)
            ones_sb = const.tile([128, 1], bf16)
            ones8_sb = const.tile([128, 2, 16], f8)
            one1_sb = const.tile([1, 1], f32)
            rs_sb = const.tile([1, ML], f32)
            rsf_sb = const.tile([1, MF], f32)
            rin_sb = const.tile([128, NH], f32)
            recip_sb = const.tile([128, NH], f32)
            rinf_sb = const.tile([MF, 1], f32)
            recf_sb = const.tile([MF, 1], f32)
            pf_sb = const.tile([128, NJF, MF], bf16)
            pf8_sb = const.tile([128, NJF, MF], f8)
            v0_sb = const.tile([128, D], bf16)
            outf_sb = const.tile([MF, 4, 512], f32)

            warm_sb = const.tile([1, 1], f32)
            bias_sb = const.tile([128, 1], f32)
            nc.vector.memset(bias_sb[:], -2.0)
            nc.vector.memset(ones_sb[:], 1.0)
            nc.vector.memset(ones8_sb[:], 1.0)
            nc.vector.memset(one1_sb[:], 1.0)
            # Load the Exp activation table while the PE warms up.
            nc.scalar.activation(
                out=warm_sb[:], in_=one1_sb[:],
                func=mybir.ActivationFunctionType.Exp,
            )
            nc.scalar.dma_start(out=mask_sb[:], in_=maskp[:])
            nc.scalar.dma_start(out=mask8_sb[:], in_=maskp8[:])

            # xt0 (bf16 x^T of the first FIX keys) loads on the scalar ring
            # ahead of the bounce stores: the v0 recompute consumes it
            # mid-vproj. (Not at t=0 on gpsimd - an early bulk load starves
            # the xkv/wv critical path.)
            nc.scalar.dma_start(out=xt0_sb[:], in_=xt0b[:])

            # V bounced and gathered in fp8, in two 1024-column halves
            # (each collective carries ~17us fixed overhead, so halves beat
            # quarters once the payload is fp8).
            v_bounce = [
                dram.tile([ML, 1024], f8, name=f"vb{i}") for i in range(2)
            ]
            v_ag = [
                dram.tile([S, 1024], f8, addr_space="Shared", name=f"vag{i}")
                for i in range(2)
            ]

            xt_tiles = {}

            def load_xt(tdx, eng):
                xtp = xtstream.tile([128, CC, 256], f8, tag="xt", name=f"xt{tdx}")
                eng.dma_start(out=xtp[:], in_=xt[:, :, 256 * tdx:256 * (tdx + 1)])
                xt_tiles[tdx] = xtp

            # ============ projections ============
            with (
                tc.tile_pool(name="px", bufs=1) as px,
                tc.tile_pool(name="stage", bufs=4) as stage,
                tc.tile_pool(name="proj_ps", bufs=4, space="PSUM") as proj_ps,
                tc.tile_pool(name="wvhold", bufs=4) as wvhold,
                tc.tile_pool(name="wqstream", bufs=8) as wqstream,
            ):
                xkv_sb = px.tile([128, CC, ML], bf16)
                xq_sb = px.tile([128, CC, ML], bf16)

                def load_wv(wc, wv_tiles):
                    wvt = wvhold.tile(
                        [128, CC, 256], bf16, tag="wv", name=f"wv{wc}"
                    )
                    nc.sync.dma_start(
                        out=wvt[:], in_=wv[:, :, 256 * wc:256 * (wc + 1)]
                    )
                    wv_tiles.append(wvt)

                # first-MM critical path, finely chunked: the very first
                # accumulation group touches xkv chunks + wv0 chunks in c
                # order, so 256KB of each unblocks the PE ~10us sooner than
                # whole-tile loads.
                wv_tiles = []
                wvt0 = wvhold.tile([128, CC, 256], bf16, tag="wv", name="wv0")
                nc.sync.dma_start(out=xkv_sb[:, 0:2, :], in_=xkv[:, 0:2, :])
                nc.sync.dma_start(out=wvt0[:, 0:4, :], in_=wv[:, 0:4, 0:256])
                nc.sync.dma_start(out=xkv_sb[:, 2:6, :], in_=xkv[:, 2:6, :])
                nc.sync.dma_start(out=wvt0[:, 4:CC, :], in_=wv[:, 4:CC, 0:256])
                nc.sync.dma_start(out=xkv_sb[:, 6:CC, :], in_=xkv[:, 6:CC, :])
                wv_tiles.append(wvt0)
                load_wv(1, wv_tiles)

                # ---- V projection -> fp8 bounce -> half AllGathers ----
                # Also recomputes V[0:128] in bf16 for every core (the fixup
                # needs accurate V for the earliest keys; those rows live
                # only on core 0, and collectives can't broadcast them).
                with tc.spectator_scope("vproj"):
                    for cs in range(4):
                        for half in range(2):
                            wc = 2 * cs + half
                            if wc >= 2:
                                load_wv(wc, wv_tiles)
                        for nt in range(ML // 128):
                            st = stage.tile(
                                [128, 512], f8, tag="stage",
                                name=f"vst{cs}_{nt}",
                            )
                            for half in range(2):
                                wvt = wv_tiles[2 * cs + half]
                                ps = proj_ps.tile(
                                    [128, ML], f32, tag="proj",
                                    name=f"vps{cs}_{nt}_{half}",
                                )
                                for c in range(CC):
                                    nc.tensor.matmul(
                                        out=ps[:, 0:256],
                                        lhsT=xkv_sb[:, c, 128 * nt:128 * (nt + 1)],
                                        rhs=wvt[:, c, :],
                                        start=(c == 0), stop=(c == CC - 1),
                                    )
                                nc.vector.tensor_copy(
                                    out=st[:, 256 * half:256 * (half + 1)],
                                    in_=ps[:, 0:256],
                                )
                            nc.scalar.dma_start(
                                out=v_bounce[cs // 2][128 * nt:128 * (nt + 1),
                                                     512 * (cs % 2):
                                                     512 * (cs % 2) + 512],
                                in_=st[:],
                            )
                        # bf16 V[0:128] for this column block (reuses the
                        # live wv tiles and the early-loaded xt0)
                        for half in range(2):
                            wc = 2 * cs + half
                            wvt = wv_tiles[wc]
                            ps = proj_ps.tile(
                                [128, ML], f32, tag="proj", name=f"v0ps{wc}"
                            )
                            for c in range(CC):
                                nc.tensor.matmul(
                                    out=ps[:, 0:256],
                                    lhsT=xt0_sb[:, c, 0:128],
                                    rhs=wvt[:, c, :],
                                    start=(c == 0), stop=(c == CC - 1),
                                )
                            nc.vector.tensor_copy(
                                out=v0_sb[:, 256 * wc:256 * (wc + 1)],
                                in_=ps[:, 0:256],
                            )
                        if cs % 2 == 1:
                            nc.gpsimd.collective_compute(
                                "AllGather", mybir.AluOpType.bypass,
                                replica_groups=RG,
                                ins=[v_bounce[cs // 2][:].opt()],
                                outs=[v_ag[cs // 2][:].opt()],
                            )

                # ---- Q~ projection (bf16 in, fp8 + small bf16 out) ----
                nc.sync.dma_start(out=xq_sb[:], in_=xq[:])
                wq_tiles = []

                def load_wq(dt):
                    wqt = wqstream.tile(
                        [128, CC, 128], bf16, tag="wq", name=f"wq{dt}"
                    )
                    nc.sync.dma_start(out=wqt[:], in_=wqk[dt])
                    wq_tiles.append(wqt)

                # Sync-ring order: wqk and xt tiles strictly interleaved
                # 1:1 (0.5MB each) so neither stream can starve the other's
                # consumer behind the AllGather window.
                for k in range(DT):
                    load_wq(k)
                    load_xt(k, nc.sync)
                with tc.spectator_scope("qtproj"):
                    for dt in range(DT):
                        ps = proj_ps.tile(
                            [128, ML], f32, tag="proj", name=f"qps{dt}"
                        )
                        for c in range(CC):
                            nc.tensor.matmul(
                                out=ps[:], lhsT=wq_tiles[dt][:, c, :],
                                rhs=xq_sb[:, c, :],
                                start=(c == 0), stop=(c == CC - 1),
                            )
                        nc.vector.tensor_copy(out=qt8_sb[:, dt, :], in_=ps[:])
                        nc.vector.tensor_copy(
                            out=qt16_sb[:, dt, :], in_=ps[:, 0:MF]
                        )

            # ============ attention ============
            with (
                tc.tile_pool(name="attn", bufs=1) as attn,
                tc.tile_pool(name="v8pool", bufs=16) as v8pool,
                tc.tile_pool(name="vfixp", bufs=4) as vfixp,
                tc.tile_pool(name="avstage", bufs=4) as avstage,
                tc.tile_pool(name="outp", bufs=4) as outp,
                tc.tile_pool(name="st_ps", bufs=2, space="PSUM") as st_ps,
                tc.tile_pool(name="rs_ps", bufs=1, space="PSUM") as rs_ps,
                tc.tile_pool(name="av_ps", bufs=1, space="PSUM") as av_ps,
                tc.tile_pool(name="tp_ps", bufs=1, space="PSUM") as tp_ps,
            ):
                p_all = attn.tile([128, NJ, ML], f8)
                # fp8 V rows 128:FIX per 512-column block, for the fixup's
                # later key tiles (rows 0:128 use the bf16 v0 recompute).
                # Head-blocked on each half's gather.
                vfix_tiles = []
                for ct in range(4):
                    vf = vfixp.tile(
                        [128, NJF - 1, 512], f8, tag="vfix", name=f"vfix{ct}"
                    )
                    c0 = 512 * (ct % 2)
                    nc.scalar.dma_start(
                        out=vf[:],
                        in_=v_ag[ct // 2][128:FIX, c0:c0 + 512]
                        .rearrange("(jj p) n -> p jj n", p=128),
                    )
                    vfix_tiles.append(vf)

                # ---- fixup scores: bf16 recompute of rows g < FIX ----
                with tc.spectator_scope("fixsc"):
                    for jt in range(NJF):
                        psf = st_ps.tile([128, ML], f32, tag="st")
                        for c in range(CC):
                            nc.tensor.matmul(
                                out=psf[:, 0:MF],
                                lhsT=xt0_sb[:, c, 128 * jt:128 * (jt + 1)],
                                rhs=qt16_sb[:, c, :],
                                start=(c == 0), stop=(c == CC - 1),
                            )
                        pfj = pf_sb[:, jt, :]
                        m0f = 16 * jt
                        nc.scalar.activation(
                            out=pfj[:, m0f:MF], in_=psf[:, m0f:MF], func=EXP,
                            scale=SCALE,
                        )
                        nc.vector.tensor_tensor(
                            out=pfj[:, m0f:m0f + 16], in0=pfj[:, m0f:m0f + 16],
                            in1=mask_sb[:], op=mybir.AluOpType.mult,
                        )
                        if m0f > 0:
                            nc.vector.memset(pfj[:, 0:m0f], 0.0)
                    # fixup row sums -> transpose -> reciprocal
                    rsf = rs_ps.tile([1, ML], f32, tag="rs")
                    for jt in range(NJF):
                        m0f = 16 * jt
                        nc.tensor.matmul(
                            out=rsf[0:1, m0f:MF], lhsT=ones_sb[:],
                            rhs=pf_sb[:, jt, m0f:MF],
                            start=(jt == 0), stop=(jt == NJF - 1),
                        )
                    nc.scalar.activation(
                        out=rsf_sb[:], in_=rsf[0:1, 0:MF], func=CPY
                    )
                    tpf = tp_ps.tile([MF, 1], f32, tag="tp")
                    nc.tensor.matmul(
                        out=tpf[:], lhsT=rsf_sb[:], rhs=one1_sb[:],
                        start=True, stop=True,
                    )
                    nc.scalar.activation(out=rinf_sb[:], in_=tpf[:], func=CPY)
                    nc.vector.reciprocal(out=recf_sb[:], in_=rinf_sb[:])
                    # fp8 copy of the fixup P for the fp8-V key tiles
                    nc.vector.tensor_copy(out=pf8_sb[:], in_=pf_sb[:])

                # ---- main scores: fp8, DoubleRow over contraction pairs.
                # The rowsum matmuls interleave pair-by-pair so the renorm
                # has no serial tail after the last score tile. ----
                rs = rs_ps.tile([1, ML], f32, tag="rs")
                with tc.spectator_scope("scores"):
                    for j in range(NJ):
                        tdx = j // 2
                        s0 = 128 * (j % 2)
                        kt = xt_tiles[tdx][:, :, s0:s0 + 128]
                        m0 = 16 * j
                        ps = st_ps.tile([128, ML], f32, tag="st")
                        if j < DR_J:
                            for cp in range(CC // 2):
                                nc.tensor.matmul(
                                    out=ps[:, m0:ML],
                                    lhsT=kt[:, 2 * cp:2 * cp + 2, :],
                                    rhs=qt8_sb[:, 2 * cp:2 * cp + 2, m0:ML],
                                    start=(cp == 0), stop=(cp == CC // 2 - 1),
                                    perf_mode=DR,
                                )
                        else:
                            for c in range(CC):
                                nc.tensor.matmul(
                                    out=ps[:, m0:ML], lhsT=kt[:, c, :],
                                    rhs=qt8_sb[:, c, m0:ML],
                                    start=(c == 0), stop=(c == CC - 1),
                                )
                        pj = p_all[:, j, :]
                        # -2 bias keeps exp(s) under the e4m3 max (240); it
                        # cancels in the row normalization.
                        nc.scalar.activation(
                            out=pj[:, m0:ML], in_=ps[:, m0:ML], func=EXP,
                            scale=SCALE, bias=bias_sb[:],
                        )
                        nc.vector.tensor_tensor(
                            out=pj[:, m0:m0 + 16], in0=pj[:, m0:m0 + 16],
                            in1=mask8_sb[:], op=mybir.AluOpType.mult,
                        )
                        g0 = 128 * (j // 8)
                        if m0 > g0:
                            nc.vector.memset(pj[:, g0:m0], 0.0)
                        if j % 2 == 1:
                            j0 = j - 1
                            m0p = 16 * j0
                            nc.tensor.matmul(
                                out=rs[0:1, m0p:ML], lhsT=ones8_sb[:, :, 0:1],
                                rhs=p_all[:, j0:j0 + 2, m0p:ML],
                                start=(j0 == 0), stop=(j0 == NJ - 2),
                                perf_mode=DR,
                            )


                # ---- fixup AV: after the scores so its ACT drains sit
                # behind the exp stream (they gate only the main AV's first
                # PSUM bank, which starts later anyway) ----
                with tc.spectator_scope("fixav"):
                    for cs in range(4):
                        av_f = av_ps.tile(
                            [128, 512], f32, tag="av0", name=f"avf_{cs}"
                        )
                        nc.tensor.matmul(
                            out=av_f[0:MF, :], lhsT=pf_sb[:, 0, :],
                            rhs=v0_sb[:, 512 * cs:512 * (cs + 1)],
                            start=True, stop=False,
                        )
                        for jt in range(1, NJF):
                            nc.tensor.matmul(
                                out=av_f[0:MF, :], lhsT=pf8_sb[:, jt, :],
                                rhs=vfix_tiles[cs][:, jt - 1, :],
                                start=False, stop=(jt == NJF - 1),
                            )
                        nc.scalar.activation(
                            out=outf_sb[:, cs, :], in_=av_f[0:MF, :], func=CPY,
                            scale=recf_sb[:],
                        )

                with tc.spectator_scope("renorm"):
                    nc.scalar.activation(out=rs_sb[:], in_=rs[:], func=CPY)
                    for h in range(NH):
                        tp = tp_ps.tile([128, 1], f32, tag="tp")
                        nc.tensor.matmul(
                            out=tp[:], lhsT=rs_sb[0:1, 128 * h:128 * (h + 1)],
                            rhs=one1_sb[:], start=True, stop=True,
                        )
                        nc.scalar.activation(
                            out=rin_sb[:, h:h + 1], in_=tp[:], func=CPY
                        )
                    nc.vector.reciprocal(out=recip_sb[:], in_=rin_sb[:])

                with tc.spectator_scope("av"):
                    for cs in range(4):
                        av = [
                            av_ps.tile([128, 512], f32, tag=f"av{h}",
                                       name=f"av{h}_{cs}")
                            for h in range(NH)
                        ]
                        c0 = 512 * (cs % 2)
                        for t in range((NJ + 3) // 4):
                            vt8 = v8pool.tile([128, 4, 512], f8, tag="v8")
                            nc.sync.dma_start(
                                out=vt8[:],
                                in_=v_ag[cs // 2][512 * t:512 * (t + 1),
                                                  c0:c0 + 512]
                                .rearrange("(jj p) n -> p jj n", p=128),
                            )
                            for jj0 in (0, 2):
                                j0 = 4 * t + jj0
                                for h in range(j0 // 8, NH):
                                    nc.tensor.matmul(
                                        out=av[h][:],
                                        lhsT=p_all[:, j0:j0 + 2,
                                                   128 * h:128 * (h + 1)],
                                        rhs=vt8[:, jj0:jj0 + 2, :],
                                        start=(j0 == 0),
                                        stop=(j0 + 1 == min(8 * (h + 1), NJ) - 1),
                                        perf_mode=DR,
                                    )
                            if t % 2 == 1:
                                h = (t - 1) // 2
                                stg = avstage.tile(
                                    [128, 512], f32, tag="avs",
                                    name=f"avs{h}_{cs}"
                                )
                                nc.vector.tensor_copy(out=stg[:], in_=av[h][:])
                                ob = outp.tile([128, 512], f32, tag="out")
                                nc.scalar.activation(
                                    out=ob[:], in_=stg[:], func=CPY,
                                    scale=recip_sb[:, h:h + 1],
                                )
                                if h == 0:
                                    # overwrite the fixed-up early rows
                                    nc.vector.tensor_copy(
                                        out=ob[0:MF, :], in_=outf_sb[:, cs, :]
                                    )
                                nc.scalar.dma_start(
                                    out=out[128 * h:128 * (h + 1),
                                            512 * cs:512 * (cs + 1)],
                                    in_=ob[:],
                                )

    nc.finalize()
    return nc


def _prep_inputs(x, Wq, Wk, Wv, S):
    import ml_dtypes

    bf = ml_dtypes.bfloat16
    f8 = ml_dtypes.float8_e4m3
    ML = S // N_CORES

    def shuf_w(W):
        # [dt, p, c, j] layout: element = W[128c+p, 128dt+j]
        return np.ascontiguousarray(
            W.reshape(CC, 128, DT, 128).transpose(2, 1, 0, 3)
        ).astype(bf)

    wqk_h = shuf_w((Wq @ Wk.T).astype(np.float32))
    wv_h = np.ascontiguousarray(
        Wv.reshape(CC, 128, D).transpose(1, 0, 2)
    ).astype(bf)

    def shuf_x(rows, dt):
        # rows [n, D] -> [p, c, m] with element = rows[m, 128c+p]
        n = rows.shape[0]
        return np.ascontiguousarray(
            rows.reshape(n, CC, 128).transpose(2, 1, 0)
        ).astype(dt)

    xt_h = shuf_x(x, f8)
    xt0b_h = shuf_x(x[0:FIX], bf)
    in_maps = []
    for i in range(N_CORES):
        mask = (np.arange(128)[:, None] <= 8 * np.arange(16)[None, :] + i)
        in_maps.append({
            "xq": shuf_x(x[i::N_CORES], bf),
            "xkv": shuf_x(x[ML * i:ML * (i + 1)], bf),
            "xt": xt_h,
            "xt0b": xt0b_h,
            "wqk": wqk_h, "wv": wv_h,
            "mask": mask.astype(bf),
            "mask8": mask.astype(f8),
        })
    return in_maps


def run(x, Wq, Wk, Wv, S, trace=False, trace_cores=None):
    from concourse.bass_utils import run_bass_kernel_spmd

    if S not in _BUILT:
        _BUILT[S] = _build(S)
    nc = _BUILT[S]
    in_maps = _prep_inputs(x, Wq, Wk, Wv, S)
    res = run_bass_kernel_spmd(
        nc, in_maps, list(range(N_CORES)), trace=trace, trace_cores=trace_cores
    )
    outs = [res.results[i]["out"] for i in range(N_CORES)]
    full = np.stack(outs, axis=1).reshape(S, D).astype(np.float32)
    return full, res


def kernel(x, Wq, Wk, Wv):
    x = np.asarray(x, dtype=np.float32)
    Wq = np.asarray(Wq, dtype=np.float32)
    Wk = np.asarray(Wk, dtype=np.float32)
    Wv = np.asarray(Wv, dtype=np.float32)
    full, _ = run(x, Wq, Wk, Wv, x.shape[0])
    return full
